# revision 22
# baseline (speedup 1.0000x reference)
"""GCN encoder (Linear+ReLU -> GCNConv+ReLU -> GCNConv -> ReLU) on 8 TRN2
NeuronCores.

Architecture (v7): fully node-sharded with two 8-rank AllGathers.
  - Core c computes z1 = dinv*(relu(x_c @ fc_W + fc_b) @ W1) for its own
    2500 nodes only (~30us of PE) and AllGathers the bf16 table.  The
    collective-runtime barrier (~40-65us warm) elapses during the early
    compute, so AG1 starts almost immediately after z1.
  - Degrees come from a single DVE reduce over a compact host layout of
    the own-shard edge weights (w at [dst%128, dst//128, k]) - no
    one-hot matmul pass and no cross-core exchange.
  - Per-chunk tile segments [self][local][remote].  Self rows are read
    with a static DMA from the own-shard table (identity stationary, no
    selw entry, no gather).  Local (in-shard src) tiles gather from the
    own table before the AllGather lands; remote tiles gather from the
    AllGathered table.  The same index arrays serve both layers.
  - z2 is produced per-chunk inside the layer-1 loop; during the z2
    AllGather the layer-2 self/local partial sums accumulate into SBUF.

Host-side preprocessing is index manipulation / data layout only.  All
arithmetic (degree sums, rsqrt, matmuls, aggregation) runs on device.
"""

import os

import numpy as np
import ml_dtypes

import concourse.bacc as bacc
import concourse.bass as bass
import concourse.mybir as mybir
import concourse.tile as tile
from concourse.bass_utils import run_bass_kernel_spmd
from concourse.masks import make_identity

F32 = mybir.dt.float32
BF16 = mybir.dt.bfloat16
I16 = mybir.dt.int16

N = 20000
E = 320000
IN_FT, HID1, HID2, OUT_FT = 256, 400, 200, 128
NCORES = 8
SHARD = N // NCORES            # 2500 nodes per core
NCH = (SHARD + 127) // 128     # 20 local dst chunks per core (last 68)
TAB1_W = 256                   # padded row width of layer-1 gather table
AluOp = mybir.AluOpType
ActFn = mybir.ActivationFunctionType


def _cdiv(a, b):
    return (a + b - 1) // b


# --------------------------------------------------------------------------
# Host-side sharding / layout
# --------------------------------------------------------------------------

def _idx_layout(a):
    g = a.astype(np.int16).reshape(-1, 16).T.copy()
    return np.ascontiguousarray(np.tile(g, (8, 1)))


def _prep_edges(edge_index, edge_attr):
    """Partition edges by dst shard into per-chunk segments
    [local(in-shard src)][remote], padded to 128-edge tiles with counts
    maximized across cores (one SPMD program serves all cores).  Self
    loops are NOT in the edge lists (identity-stationary on the table
    chunk rows).  selw covers local+remote tiles."""
    src = np.ascontiguousarray(edge_index[0]).astype(np.int64)
    dst = np.ascontiguousarray(edge_index[1]).astype(np.int64)
    w_all = np.ascontiguousarray(edge_attr).astype(np.float32)

    per_core = []
    ecounts = np.zeros((NCORES, NCH), np.int64)
    for c in range(NCORES):
        lo, hi = c * SHARD, (c + 1) * SHARD
        m = (dst >= lo) & (dst < hi)
        s = src[m]
        d = dst[m] - lo
        w = w_all[m]
        o = np.argsort(d >> 7, kind="stable")
        s, d, w = s[o], d[o], w[o]
        ch = d >> 7
        for j in range(NCH):
            ecounts[c, j] = int((ch == j).sum())
        per_core.append((s, d, w, lo))

    etiles = [max(1, _cdiv(int(ecounts[:, j].max()), 128))
              for j in range(NCH)]
    net = int(np.sum(etiles))
    nsel = net
    ep = 128 * nsel
    eoff = np.concatenate([[0], np.cumsum(etiles)])[:-1]

    # per-core own-degree layout sized by the global max per-dst count
    K2 = 0
    for c in range(NCORES):
        lo = c * SHARD
        m = (dst >= lo) & (dst < lo + SHARD)
        dl = dst[m] - lo
        cnt = np.zeros((NCH, 128), np.int64)
        np.add.at(cnt, (dl >> 7, dl & 127), 1)
        K2 = max(K2, int(cnt.max()))

    in_edges = []
    for c in range(NCORES):
        s, d, w, lo = per_core[c]
        ch = d >> 7
        w_pad = np.zeros(ep, np.float32)
        dslot_pad = np.zeros(ep, np.int64)
        esrc = np.zeros(128 * net, np.int64)
        for j in range(NCH):
            mm = ch == j
            se, de, we = s[mm], d[mm], w[mm]
            ge = 128 * int(eoff[j])
            dslot_pad[ge:ge + len(se)] = de - 128 * j
            w_pad[ge:ge + len(se)] = we
            esrc[ge:ge + len(se)] = se          # global rows
        selw = np.zeros((ep, 128), np.float32)
        selw[np.arange(ep), dslot_pad] = w_pad
        selw_pack = (
            selw.reshape(nsel, 128, 128)
            .transpose(1, 0, 2)
            .reshape(128, nsel * 128)
            .astype(ml_dtypes.bfloat16)
        )
        degw_own = np.zeros((128, NCH, K2), np.float32)
        m = (dst >= lo) & (dst < lo + SHARD)
        dl_all = dst[m] - lo
        wl_all = w_all[m]
        kfill2 = np.zeros((NCH, 128), np.int64)
        lp, lc = dl_all & 127, dl_all >> 7
        for i in range(len(dl_all)):
            p, chn = int(lp[i]), int(lc[i])
            degw_own[p, chn, kfill2[chn, p]] = wl_all[i]
            kfill2[chn, p] += 1

        in_edges.append({
            "selw": selw_pack,
            "egidx": _idx_layout(esrc),
            "degw_own": degw_own.reshape(128, -1),
        })
    meta = dict(etiles=etiles, eoff=eoff, net=net, K2=K2)
    return meta, ep, in_edges


# --------------------------------------------------------------------------
# Device program
# --------------------------------------------------------------------------

def _fix_multiwait(nc):
    """This neuronxcc build only accepts ONE sync-wait on non-EventSemaphore
    instructions; bacc's splitter allows two on DMAs.  Move excess waits onto
    inserted EventSemaphore NOPs (2 waits each) preceding the instruction."""
    nev = 0
    for bb in nc.main_func.blocks:
        changed = False
        out = []
        for ins in bb.instructions:
            si = ins.sync_info
            waits = list(si.on_wait) if si and si.on_wait else []
            limit = 2 if isinstance(ins, mybir.InstEventSemaphore) else 1
            if len(waits) > limit:
                extra, keep = waits[:-limit], waits[-limit:]
                for i in range(0, len(extra), 2):
                    ev = mybir.InstEventSemaphore(
                        name=f"{ins.name}-evw{i}", ins=[], outs=[])
                    ev.engine = ins.engine
                    ev.sync_info = mybir.SyncInfo(
                        on_wait=extra[i:i + 2], on_update=[])
                    out.append(ev)
                    nev += 1
                si.on_wait = keep
                changed = True
            out.append(ins)
        if changed:
            bb.instructions = out
    return nev


def _dummy_out(nc, wpool, out_d):
    for j in range(NCH):
        cw = min(128, SHARD - 128 * j)
        o_sb = wpool.tile([128, OUT_FT], F32, tag="osb")
        nc.vector.memset(o_sb[:], 0.0)
        nc.sync.dma_start(out=out_d[128 * j:128 * j + cw, :],
                          in_=o_sb[:cw, :])


def build_nc(meta, ep):
    stage = int(os.environ.get("K_STAGE", "50"))
    etiles, eoff = meta["etiles"], meta["eoff"]
    net, K2 = meta["net"], meta["K2"]
    nsel = net
    assert ep == 128 * nsel
    nc = bacc.Bacc("TRN2", target_bir_lowering=False, debug=False,
                   num_devices=NCORES, num_swdge_queues=4)

    xt_d = nc.dram_tensor("xt", [IN_FT, SHARD], BF16,
                          kind="ExternalInput")
    selw_d = nc.dram_tensor("selw", [128, ep], BF16, kind="ExternalInput")
    egidx_d = nc.dram_tensor("egidx", [128, 8 * net], I16,
                             kind="ExternalInput")
    degwo_d = nc.dram_tensor("degw_own", [128, NCH * K2], F32,
                             kind="ExternalInput")
    fcw_d = nc.dram_tensor("fcw", [IN_FT, HID1], BF16,
                           kind="ExternalInput")
    fcb_d = nc.dram_tensor("fcb", [HID1, 1], F32, kind="ExternalInput")
    w1_d = nc.dram_tensor("w1", [HID1, HID2], BF16, kind="ExternalInput")
    b1_d = nc.dram_tensor("b1", [1, HID2], BF16, kind="ExternalInput")
    w2_d = nc.dram_tensor("w2", [HID2, OUT_FT], BF16,
                          kind="ExternalInput")
    b2_d = nc.dram_tensor("b2", [1, OUT_FT], BF16, kind="ExternalInput")
    out_d = nc.dram_tensor("out", [SHARD, OUT_FT], F32, kind="ExternalOutput")

    n_fi = _cdiv(IN_FT, 128)     # 2
    n_fo = _cdiv(HID1, 128)      # 4 (128,128,128,16)
    n_k2 = _cdiv(HID2, 128)      # 2 (128,72)
    fo_sizes = [min(128, HID1 - 128 * i) for i in range(n_fo)]
    k2_sizes = [min(128, HID2 - 128 * i) for i in range(n_k2)]
    NSUB = 5
    SUB = SHARD // NSUB          # 500

    def _emit(tc, cpool, apool, gpool, wpool, psA, psB, psT, dpool):
        # ---------------- early inputs ----------------
        degwo_sb = cpool.tile([128, NCH * K2], F32)
        nc.sync.dma_start(out=degwo_sb[:], in_=degwo_d[:])

        fcw_sb = []
        for i in range(n_fi):
            t = cpool.tile([128, HID1], BF16, name=f"fcw{i}")
            nc.scalar.dma_start(out=t[:],
                                in_=fcw_d[128 * i:128 * (i + 1), :])
            fcw_sb.append(t)
        w1_sb = []
        for i in range(n_fo):
            t = cpool.tile([fo_sizes[i], HID2], BF16, name=f"w1_{i}")
            nc.scalar.dma_start(
                out=t[:], in_=w1_d[128 * i:128 * i + fo_sizes[i], :])
            w1_sb.append(t)
        w2_sb = []
        for i in range(n_k2):
            t = cpool.tile([k2_sizes[i], OUT_FT], BF16, name=f"w2_{i}")
            nc.scalar.dma_start(
                out=t[:], in_=w2_d[128 * i:128 * i + k2_sizes[i], :])
            w2_sb.append(t)
        fcb_sb = cpool.tile([128, n_fo], F32, name="fcb_sb")
        for i in range(n_fo):
            nc.sync.dma_start(
                out=fcb_sb[:fo_sizes[i], i:i + 1],
                in_=fcb_d[128 * i:128 * i + fo_sizes[i], :])
        b1_sb = cpool.tile([1, HID2], BF16)
        nc.scalar.dma_start(out=b1_sb[:], in_=b1_d[:])
        b2_sb = cpool.tile([1, OUT_FT], BF16)
        nc.scalar.dma_start(out=b2_sb[:], in_=b2_d[:])

        ident = cpool.tile([128, 128], BF16)
        make_identity(nc, ident[:])

        # ---------------- degrees / normalization (DVE) ---------------
        deg_own = cpool.tile([128, NCH], F32)
        nc.vector.tensor_reduce(
            out=deg_own[:],
            in_=degwo_sb[:].rearrange("p (c k) -> p c k", k=K2),
            axis=mybir.AxisListType.X, op=AluOp.add)
        nc.vector.tensor_scalar_add(deg_own[:], deg_own[:], 1.0)
        dinv_own = cpool.tile([128, NCH], F32)
        nc.vector.reciprocal(out=dinv_own[:], in_=deg_own[:])
        nc.scalar.activation(out=dinv_own[:], in_=dinv_own[:],
                             func=ActFn.Sqrt)
        sqd_own = cpool.tile([128, NCH], BF16)
        nc.scalar.activation(out=sqd_own[:], in_=deg_own[:],
                             func=ActFn.Sqrt)

        # selw / idx loads stream in behind the small early loads
        selw_sb = cpool.tile([128, ep], BF16)
        for s0 in range(0, nsel, 44):
            s1 = min(nsel, s0 + 44)
            nc.sync.dma_start(out=selw_sb[:, 128 * s0:128 * s1],
                              in_=selw_d[:, 128 * s0:128 * s1])
        egidx_sb = cpool.tile([128, 8 * net], I16)
        nc.sync.dma_start(out=egidx_sb[:], in_=egidx_d[:])

        if stage < 12:
            _dummy_out(nc, wpool, out_d)
            return

        # ---------------- phase A + z1 (own shard) -> own table -------
        ztab1_mine = dpool.tile([SHARD, TAB1_W], BF16)
        h0strip = []
        for i in range(n_fo):
            t_h = apool.tile([fo_sizes[i], SHARD], BF16,
                             name=f"h0strip{i}")
            h0strip.append(t_h)
        with tc.tile_pool(name="phA", bufs=2) as tpool:
            nxt_ch = 0
            for s in range(NSUB):
                xts = []
                for k in range(n_fi):
                    xk = tpool.tile([128, SUB], BF16, tag="xts",
                                    name=f"xts{k}", bufs=3)
                    nc.scalar.dma_start(
                        out=xk[:],
                        in_=xt_d[128 * k:128 * (k + 1),
                                 SUB * s:SUB * (s + 1)])
                    xts.append(xk)
                for i in range(n_fo):
                    ps_h = psA.tile([fo_sizes[i], SUB], F32, tag="ph")
                    for k in range(n_fi):
                        nc.tensor.matmul(
                            out=ps_h[:],
                            lhsT=fcw_sb[k][:, 128 * i:128 * i
                                           + fo_sizes[i]],
                            rhs=xts[k][:],
                            start=(k == 0), stop=(k == n_fi - 1),
                        )
                    nc.vector.tensor_scalar(
                        out=h0strip[i][:, SUB * s:SUB * (s + 1)],
                        in0=ps_h[:],
                        scalar1=fcb_sb[:fo_sizes[i], i:i + 1],
                        scalar2=0.0,
                        op0=AluOp.add, op1=AluOp.max,
                    )
                end = SUB * (s + 1)
                while (nxt_ch + 1) * 128 <= end or (
                        s == NSUB - 1 and nxt_ch < NCH):
                    ch = nxt_ch
                    nxt_ch += 1
                    cw = min(128, SHARD - 128 * ch)
                    ps_z = psB.tile([128, HID2], F32, tag="b")
                    for i in range(n_fo):
                        nc.tensor.matmul(
                            out=ps_z[:cw, :],
                            lhsT=h0strip[i][:, 128 * ch:128 * ch + cw],
                            rhs=w1_sb[i][:],
                            start=(i == 0), stop=(i == n_fo - 1),
                        )
                    zrow = tpool.tile([128, TAB1_W], BF16, tag="zrow",
                                      name="zrow", bufs=3)
                    nc.scalar.mul(out=zrow[:cw, :HID2], in_=ps_z[:cw, :],
                                  mul=dinv_own[:cw, ch:ch + 1])
                    nc.sync.dma_start(
                        out=ztab1_mine[128 * ch:128 * ch + cw, :],
                        in_=zrow[:cw, :])

        rg = [list(range(NCORES))]
        ztab1_full = nc.dram_tensor("ztab1_full", [N, TAB1_W], BF16,
                                    addr_space="Shared")
        cc1i = nc.gpsimd.collective_compute(
            "AllGather", AluOp.bypass, replica_groups=rg,
            ins=[ztab1_mine.opt()], outs=[ztab1_full.ap()[:]],
        )
        cc1 = [cc1i.ins]

        # deferred: sqd row layout (PE op; avoid head-of-line pre-phA)
        ps_tr = psT.tile([NCH, 128], BF16, tag="tr")
        nc.tensor.transpose(out=ps_tr[:], in_=sqd_own[:],
                            identity=ident[:])
        sqd_rows = cpool.tile([NCH, 128], BF16)
        nc.vector.tensor_copy(out=sqd_rows[:], in_=ps_tr[:])
        sqdT = cpool.tile([1, 128 * NCH], BF16)
        for j in range(NCH):
            nc.sync.dma_start(out=sqdT[:, 128 * j:128 * (j + 1)],
                              in_=sqd_rows[j:j + 1, :])

        if stage < 14:
            _dummy_out(nc, wpool, out_d)
            return

        # ---------------- gather helper -------------------------------
        def emit_gather(gtiles, table, width, ccdeps, idx_sb, seq_tiles,
                        tag, bufs):
            g = len(gtiles)
            t0 = 8 * g
            nt = min(8, seq_tiles - t0)
            graw = gpool.tile([128, 8 * width], BF16, tag=tag,
                              name=f"g{tag}{width}_{g}", bufs=bufs)
            sub = graw[:, :nt * width].rearrange("p (t f) -> p t f",
                                                 f=width)
            if isinstance(table, bass.DRamTensorHandle):
                table_ap = table.ap()
            elif isinstance(table, bass.AP):
                table_ap = table
            else:
                table_ap = table[:]
            gi = nc.gpsimd.dma_gather(
                sub, table_ap, idx_sb[:, 8 * t0:8 * (t0 + nt)],
                nt * 128, nt * 128, width, queue_num=g % 4)
            for cc in ccdeps:
                tile.add_dep_helper(gi.ins, cc,
                                    reason="gather reads AllGather table")
            gtiles.append(graw)

        # ---------------- layer 1 + z2 --------------------------------
        l1_sb = apool.tile([128, NCH, HID2], BF16)
        ztab2_mine = dpool.tile([SHARD, OUT_FT], BF16)
        rg1 = []
        while len(rg1) * 8 < net:
            emit_gather(rg1, ztab1_full, TAB1_W, cc1, egidx_sb, net,
                        "gr1", 8)

        for j in range(NCH):
            cw = min(128, SHARD - 128 * j)
            zself = wpool.tile([128, TAB1_W], BF16, tag="zself1",
                               bufs=2)
            nc.sync.dma_start(
                out=zself[:cw, :],
                in_=ztab1_mine[128 * j:128 * j + cw, :])
            ps_a = psB.tile([128, HID2], F32, tag="b")
            for t in range(etiles[j]):
                seq = int(eoff[j]) + t
                nc.tensor.matmul(
                    out=ps_a[:],
                    lhsT=selw_sb[:, 128 * seq:128 * (seq + 1)],
                    rhs=rg1[seq // 8][:, (seq % 8) * TAB1_W:
                                      (seq % 8) * TAB1_W + HID2],
                    start=(t == 0), stop=False,
                )
            nc.tensor.matmul(
                out=ps_a[:], lhsT=ident[:cw, :],
                rhs=zself[:cw, :HID2],
                start=False, stop=False,
            )
            nc.tensor.matmul(
                out=ps_a[:],
                lhsT=sqdT[:, 128 * j:128 * (j + 1)],
                rhs=b1_sb[:], start=False, stop=True,
            )
            nc.scalar.activation(out=l1_sb[:, j, :], in_=ps_a[:],
                                 func=ActFn.Relu,
                                 scale=dinv_own[:, j:j + 1])
            # ---- z2 for chunk j (interleaved) ----
            l1T = []
            for i in range(n_k2):
                ps_tr2 = psT.tile([128, 128], BF16, tag="tr")
                nc.tensor.transpose(
                    out=ps_tr2[:k2_sizes[i], :],
                    in_=l1_sb[:, j, 128 * i:128 * i + k2_sizes[i]],
                    identity=ident[:],
                )
                lt2 = wpool.tile([128, 128], BF16, tag="l1T")
                nc.vector.tensor_copy(out=lt2[:k2_sizes[i], :],
                                      in_=ps_tr2[:k2_sizes[i], :])
                l1T.append(lt2)
            ps_z2 = psB.tile([128, OUT_FT], F32, tag="b")
            for i in range(n_k2):
                nc.tensor.matmul(
                    out=ps_z2[:],
                    lhsT=l1T[i][:k2_sizes[i], :],
                    rhs=w2_sb[i][:],
                    start=(i == 0), stop=(i == n_k2 - 1),
                )
            zrow2 = wpool.tile([128, OUT_FT], BF16, tag="zrow2",
                               bufs=3)
            nc.scalar.mul(out=zrow2[:], in_=ps_z2[:],
                          mul=dinv_own[:, j:j + 1])
            nc.sync.dma_start(
                out=ztab2_mine[128 * j:128 * j + cw, :],
                in_=zrow2[:cw, :])

        if stage < 40:
            for j in range(NCH):
                cw = min(128, SHARD - 128 * j)
                o_sb = wpool.tile([128, OUT_FT], F32, tag="osb")
                nc.scalar.copy(out=o_sb[:], in_=l1_sb[:, j, :OUT_FT])
                nc.sync.dma_start(
                    out=out_d[128 * j:128 * j + cw, :],
                    in_=o_sb[:cw, :])
            return

        ztab2_full = nc.dram_tensor("ztab2_full", [N, OUT_FT], BF16,
                                    addr_space="Shared")
        cc2i = nc.gpsimd.collective_compute(
            "AllGather", AluOp.bypass, replica_groups=rg,
            ins=[ztab2_mine.opt()], outs=[ztab2_full.ap()[:]],
        )
        cc2 = [cc2i.ins]

        if stage < 50:
            _dummy_out(nc, wpool, out_d)
            return

        # ---------------- layer-2 aggregation -------------------------
        # Pass 1 (overlaps the AllGather): self rows via static DMA
        # from ztab2_mine + local-src gathers; partial sums -> SBUF.
        l2acc = apool.tile([128, NCH, OUT_FT], BF16)
        for j in range(NCH):
            cw = min(128, SHARD - 128 * j)
            zself = wpool.tile([128, OUT_FT], BF16, tag="zself",
                               bufs=2)
            nc.sync.dma_start(
                out=zself[:cw, :],
                in_=ztab2_mine[128 * j:128 * j + cw, :])
            ps_l = psB.tile([128, OUT_FT], F32, tag="b")
            nc.tensor.matmul(
                out=ps_l[:], lhsT=ident[:cw, :], rhs=zself[:cw, :],
                start=True, stop=False,
            )
            nc.tensor.matmul(
                out=ps_l[:],
                lhsT=sqdT[:, 128 * j:128 * (j + 1)],
                rhs=b2_sb[:], start=False, stop=True,
            )
            nc.vector.tensor_copy(out=l2acc[:, j, :], in_=ps_l[:])

        # Pass 2: the edge-tile stream from the AllGathered table.
        rg2 = []
        for j in range(NCH):
            et_ = etiles[j]
            cw = min(128, SHARD - 128 * j)
            while len(rg2) * 8 < int(eoff[j]) + et_:
                emit_gather(rg2, ztab2_full, OUT_FT, cc2, egidx_sb,
                            net, "gr2", 8)
            ps_a2 = psB.tile([128, OUT_FT], F32, tag="b")
            for t in range(et_):
                seq = int(eoff[j]) + t
                nc.tensor.matmul(
                    out=ps_a2[:],
                    lhsT=selw_sb[:, 128 * seq:128 * (seq + 1)],
                    rhs=rg2[seq // 8][:, (seq % 8) * OUT_FT:
                                      (seq % 8 + 1) * OUT_FT],
                    start=(t == 0), stop=(t == et_ - 1),
                )
            o_f32 = wpool.tile([128, OUT_FT], F32, tag="of")
            nc.vector.tensor_tensor(
                out=o_f32[:], in0=ps_a2[:], in1=l2acc[:, j, :],
                op=AluOp.add)
            o_sb = wpool.tile([128, OUT_FT], F32, tag="osb")
            nc.scalar.activation(out=o_sb[:], in_=o_f32[:],
                                 func=ActFn.Relu,
                                 scale=dinv_own[:, j:j + 1])
            nc.sync.dma_start(out=out_d[128 * j:128 * j + cw, :],
                              in_=o_sb[:cw, :])

    with tile.TileContext(nc) as tc:
        with (
            tc.tile_pool(name="const", bufs=1) as cpool,
            tc.tile_pool(name="acts", bufs=1) as apool,
            tc.tile_pool(name="gath", bufs=1) as gpool,
            tc.tile_pool(name="work", bufs=2) as wpool,
            tc.tile_pool(name="psA", bufs=2, space="PSUM") as psA,
            tc.tile_pool(name="psB", bufs=2, space="PSUM") as psB,
            tc.tile_pool(name="psT", bufs=2, space="PSUM") as psT,
            tc.tile_pool(name="dram", bufs=1, space="DRAM") as dpool,
        ):
            _emit(tc, cpool, apool, gpool, wpool, psA, psB, psT, dpool)
    nc.compile()
    _fix_multiwait(nc)
    return nc


# --------------------------------------------------------------------------
# Entry point
# --------------------------------------------------------------------------

_NC_CACHE = {}


def kernel(x, edge_index, edge_attr, fc_W, fc_b, W1, b1, W2, b2,
           _trace=False):
    meta, ep, in_edges = _prep_edges(edge_index, edge_attr)
    key = (tuple(meta["etiles"]), meta["K2"])
    if key not in _NC_CACHE:
        _NC_CACHE[key] = build_nc(meta, ep)
    nc = _NC_CACHE[key]

    x = np.asarray(x, np.float32)
    bf = ml_dtypes.bfloat16
    shared = {
        "fcw": np.asarray(fc_W, np.float32).astype(bf),
        "fcb": np.asarray(fc_b, np.float32).reshape(HID1, 1),
        "w1": np.asarray(W1, np.float32).astype(bf),
        "b1": np.asarray(b1, np.float32).reshape(1, HID2).astype(bf),
        "w2": np.asarray(W2, np.float32).astype(bf),
        "b2": np.asarray(b2, np.float32).reshape(1, OUT_FT).astype(bf),
    }
    in_maps = []
    for c in range(NCORES):
        xt = np.ascontiguousarray(
            x[c * SHARD:(c + 1) * SHARD, :].T).astype(bf)
        in_maps.append({"xt": xt, **in_edges[c], **shared})

    res = run_bass_kernel_spmd(nc, in_maps, list(range(NCORES)),
                               trace=_trace)
    out = np.concatenate([res.results[c]["out"] for c in range(NCORES)],
                         axis=0)
    if _trace:
        kernel._last_exec_time_ns = res.exec_time_ns
        kernel._last_results = res
    return out


# revision 23
# speedup vs baseline: 1.0113x; 1.0113x over previous
"""GCN encoder (Linear+ReLU -> GCNConv+ReLU -> GCNConv -> ReLU) on 8 TRN2
NeuronCores.

Architecture (v7): fully node-sharded with two 8-rank AllGathers.
  - Core c computes z1 = dinv*(relu(x_c @ fc_W + fc_b) @ W1) for its own
    2500 nodes only (~30us of PE) and AllGathers the bf16 table.  The
    collective-runtime barrier (~40-65us warm) elapses during the early
    compute, so AG1 starts almost immediately after z1.
  - Degrees come from a single DVE reduce over a compact host layout of
    the own-shard edge weights (w at [dst%128, dst//128, k]) - no
    one-hot matmul pass and no cross-core exchange.
  - Per-chunk tile segments [self][local][remote].  Self rows are read
    with a static DMA from the own-shard table (identity stationary, no
    selw entry, no gather).  Local (in-shard src) tiles gather from the
    own table before the AllGather lands; remote tiles gather from the
    AllGathered table.  The same index arrays serve both layers.
  - z2 is produced per-chunk inside the layer-1 loop; during the z2
    AllGather the layer-2 self/local partial sums accumulate into SBUF.

Host-side preprocessing is index manipulation / data layout only.  All
arithmetic (degree sums, rsqrt, matmuls, aggregation) runs on device.
"""

import os

import numpy as np
import ml_dtypes

import concourse.bacc as bacc
import concourse.bass as bass
import concourse.mybir as mybir
import concourse.tile as tile
from concourse.bass_utils import run_bass_kernel_spmd
from concourse.masks import make_identity

F32 = mybir.dt.float32
BF16 = mybir.dt.bfloat16
I16 = mybir.dt.int16

N = 20000
E = 320000
IN_FT, HID1, HID2, OUT_FT = 256, 400, 200, 128
NCORES = 8
SHARD = N // NCORES            # 2500 nodes per core
NCH = (SHARD + 127) // 128     # 20 local dst chunks per core (last 68)
TAB1_W = 256                   # padded row width of layer-1 gather table
AluOp = mybir.AluOpType
ActFn = mybir.ActivationFunctionType


def _cdiv(a, b):
    return (a + b - 1) // b


# --------------------------------------------------------------------------
# Host-side sharding / layout
# --------------------------------------------------------------------------

def _idx_layout(a):
    g = a.astype(np.int16).reshape(-1, 16).T.copy()
    return np.ascontiguousarray(np.tile(g, (8, 1)))


def _prep_edges(edge_index, edge_attr):
    """Partition edges by dst shard into per-chunk segments
    [local(in-shard src)][remote], padded to 128-edge tiles with counts
    maximized across cores (one SPMD program serves all cores).  Self
    loops are NOT in the edge lists (identity-stationary on the table
    chunk rows).  selw covers local+remote tiles."""
    src = np.ascontiguousarray(edge_index[0]).astype(np.int64)
    dst = np.ascontiguousarray(edge_index[1]).astype(np.int64)
    w_all = np.ascontiguousarray(edge_attr).astype(np.float32)

    per_core = []
    ecounts = np.zeros((NCORES, NCH), np.int64)
    for c in range(NCORES):
        lo, hi = c * SHARD, (c + 1) * SHARD
        m = (dst >= lo) & (dst < hi)
        s = src[m]
        d = dst[m] - lo
        w = w_all[m]
        o = np.argsort(d >> 7, kind="stable")
        s, d, w = s[o], d[o], w[o]
        ch = d >> 7
        for j in range(NCH):
            ecounts[c, j] = int((ch == j).sum())
        per_core.append((s, d, w, lo))

    etiles = [max(1, _cdiv(int(ecounts[:, j].max()), 128))
              for j in range(NCH)]
    net = int(np.sum(etiles))
    nsel = net
    ep = 128 * nsel
    eoff = np.concatenate([[0], np.cumsum(etiles)])[:-1]

    # per-core own-degree layout sized by the global max per-dst count
    K2 = 0
    for c in range(NCORES):
        lo = c * SHARD
        m = (dst >= lo) & (dst < lo + SHARD)
        dl = dst[m] - lo
        cnt = np.zeros((NCH, 128), np.int64)
        np.add.at(cnt, (dl >> 7, dl & 127), 1)
        K2 = max(K2, int(cnt.max()))

    in_edges = []
    for c in range(NCORES):
        s, d, w, lo = per_core[c]
        ch = d >> 7
        w_pad = np.zeros(ep, np.float32)
        dslot_pad = np.zeros(ep, np.int64)
        esrc = np.zeros(128 * net, np.int64)
        for j in range(NCH):
            mm = ch == j
            se, de, we = s[mm], d[mm], w[mm]
            ge = 128 * int(eoff[j])
            dslot_pad[ge:ge + len(se)] = de - 128 * j
            w_pad[ge:ge + len(se)] = we
            esrc[ge:ge + len(se)] = se          # global rows
        selw = np.zeros((ep, 128), np.float32)
        selw[np.arange(ep), dslot_pad] = w_pad
        selw_pack = (
            selw.reshape(nsel, 128, 128)
            .transpose(1, 0, 2)
            .reshape(128, nsel * 128)
            .astype(ml_dtypes.bfloat16)
        )
        degw_own = np.zeros((128, NCH, K2), np.float32)
        m = (dst >= lo) & (dst < lo + SHARD)
        dl_all = dst[m] - lo
        wl_all = w_all[m]
        kfill2 = np.zeros((NCH, 128), np.int64)
        lp, lc = dl_all & 127, dl_all >> 7
        for i in range(len(dl_all)):
            p, chn = int(lp[i]), int(lc[i])
            degw_own[p, chn, kfill2[chn, p]] = wl_all[i]
            kfill2[chn, p] += 1

        in_edges.append({
            "selw": selw_pack,
            "egidx": _idx_layout(esrc),
            "degw_own": degw_own.reshape(128, -1),
        })
    meta = dict(etiles=etiles, eoff=eoff, net=net, K2=K2)
    return meta, ep, in_edges


# --------------------------------------------------------------------------
# Device program
# --------------------------------------------------------------------------

def _fix_multiwait(nc):
    """This neuronxcc build only accepts ONE sync-wait on non-EventSemaphore
    instructions; bacc's splitter allows two on DMAs.  Move excess waits onto
    inserted EventSemaphore NOPs (2 waits each) preceding the instruction."""
    nev = 0
    for bb in nc.main_func.blocks:
        changed = False
        out = []
        for ins in bb.instructions:
            si = ins.sync_info
            waits = list(si.on_wait) if si and si.on_wait else []
            limit = 2 if isinstance(ins, mybir.InstEventSemaphore) else 1
            if len(waits) > limit:
                extra, keep = waits[:-limit], waits[-limit:]
                for i in range(0, len(extra), 2):
                    ev = mybir.InstEventSemaphore(
                        name=f"{ins.name}-evw{i}", ins=[], outs=[])
                    ev.engine = ins.engine
                    ev.sync_info = mybir.SyncInfo(
                        on_wait=extra[i:i + 2], on_update=[])
                    out.append(ev)
                    nev += 1
                si.on_wait = keep
                changed = True
            out.append(ins)
        if changed:
            bb.instructions = out
    return nev


def _dummy_out(nc, wpool, out_d):
    for j in range(NCH):
        cw = min(128, SHARD - 128 * j)
        o_sb = wpool.tile([128, OUT_FT], F32, tag="osb")
        nc.vector.memset(o_sb[:], 0.0)
        nc.sync.dma_start(out=out_d[128 * j:128 * j + cw, :],
                          in_=o_sb[:cw, :])


def build_nc(meta, ep):
    stage = int(os.environ.get("K_STAGE", "50"))
    etiles, eoff = meta["etiles"], meta["eoff"]
    net, K2 = meta["net"], meta["K2"]
    nsel = net
    assert ep == 128 * nsel
    nc = bacc.Bacc("TRN2", target_bir_lowering=False, debug=False,
                   num_devices=NCORES, num_swdge_queues=4)

    xt_d = nc.dram_tensor("xt", [IN_FT, SHARD], BF16,
                          kind="ExternalInput")
    selw_d = nc.dram_tensor("selw", [128, ep], BF16, kind="ExternalInput")
    egidx_d = nc.dram_tensor("egidx", [128, 8 * net], I16,
                             kind="ExternalInput")
    degwo_d = nc.dram_tensor("degw_own", [128, NCH * K2], F32,
                             kind="ExternalInput")
    fcw_d = nc.dram_tensor("fcw", [IN_FT, HID1], BF16,
                           kind="ExternalInput")
    fcb_d = nc.dram_tensor("fcb", [HID1, 1], F32, kind="ExternalInput")
    w1_d = nc.dram_tensor("w1", [HID1, HID2], BF16, kind="ExternalInput")
    b1_d = nc.dram_tensor("b1", [1, HID2], BF16, kind="ExternalInput")
    w2_d = nc.dram_tensor("w2", [HID2, OUT_FT], BF16,
                          kind="ExternalInput")
    b2_d = nc.dram_tensor("b2", [1, OUT_FT], BF16, kind="ExternalInput")
    out_d = nc.dram_tensor("out", [SHARD, OUT_FT], F32, kind="ExternalOutput")

    n_fi = _cdiv(IN_FT, 128)     # 2
    n_fo = _cdiv(HID1, 128)      # 4 (128,128,128,16)
    n_k2 = _cdiv(HID2, 128)      # 2 (128,72)
    fo_sizes = [min(128, HID1 - 128 * i) for i in range(n_fo)]
    k2_sizes = [min(128, HID2 - 128 * i) for i in range(n_k2)]
    NSUB = 5
    SUB = SHARD // NSUB          # 500

    def _emit(tc, cpool, apool, gpool, wpool, psA, psB, psT, dpool):
        # ---------------- early inputs ----------------
        degwo_sb = cpool.tile([128, NCH * K2], F32)
        nc.sync.dma_start(out=degwo_sb[:], in_=degwo_d[:])

        fcw_sb = []
        for i in range(n_fi):
            t = cpool.tile([128, HID1], BF16, name=f"fcw{i}")
            nc.scalar.dma_start(out=t[:],
                                in_=fcw_d[128 * i:128 * (i + 1), :])
            fcw_sb.append(t)
        w1_sb = []
        for i in range(n_fo):
            t = cpool.tile([fo_sizes[i], HID2], BF16, name=f"w1_{i}")
            nc.scalar.dma_start(
                out=t[:], in_=w1_d[128 * i:128 * i + fo_sizes[i], :])
            w1_sb.append(t)
        w2_sb = []
        for i in range(n_k2):
            t = cpool.tile([k2_sizes[i], OUT_FT], BF16, name=f"w2_{i}")
            nc.scalar.dma_start(
                out=t[:], in_=w2_d[128 * i:128 * i + k2_sizes[i], :])
            w2_sb.append(t)
        fcb_sb = cpool.tile([128, n_fo], F32, name="fcb_sb")
        for i in range(n_fo):
            nc.sync.dma_start(
                out=fcb_sb[:fo_sizes[i], i:i + 1],
                in_=fcb_d[128 * i:128 * i + fo_sizes[i], :])
        b1_sb = cpool.tile([1, HID2], BF16)
        nc.scalar.dma_start(out=b1_sb[:], in_=b1_d[:])
        b2_sb = cpool.tile([1, OUT_FT], BF16)
        nc.scalar.dma_start(out=b2_sb[:], in_=b2_d[:])

        ident = cpool.tile([128, 128], BF16)
        make_identity(nc, ident[:])

        # ---------------- degrees / normalization (DVE) ---------------
        deg_own = cpool.tile([128, NCH], F32)
        nc.vector.tensor_reduce(
            out=deg_own[:],
            in_=degwo_sb[:].rearrange("p (c k) -> p c k", k=K2),
            axis=mybir.AxisListType.X, op=AluOp.add)
        nc.vector.tensor_scalar_add(deg_own[:], deg_own[:], 1.0)
        dinv_own = cpool.tile([128, NCH], F32)
        nc.vector.reciprocal(out=dinv_own[:], in_=deg_own[:])
        nc.scalar.activation(out=dinv_own[:], in_=dinv_own[:],
                             func=ActFn.Sqrt)
        sqd_own = cpool.tile([128, NCH], BF16)
        nc.scalar.activation(out=sqd_own[:], in_=deg_own[:],
                             func=ActFn.Sqrt)

        # selw / idx loads stream in behind the small early loads
        selw_sb = cpool.tile([128, ep], BF16)
        for s0 in range(0, nsel, 44):
            s1 = min(nsel, s0 + 44)
            nc.sync.dma_start(out=selw_sb[:, 128 * s0:128 * s1],
                              in_=selw_d[:, 128 * s0:128 * s1])
        egidx_sb = cpool.tile([128, 8 * net], I16)
        nc.sync.dma_start(out=egidx_sb[:], in_=egidx_d[:])

        if stage < 12:
            _dummy_out(nc, wpool, out_d)
            return

        # ---------------- phase A + z1 (own shard) -> own table -------
        ztab1_mine = dpool.tile([SHARD, TAB1_W], BF16)
        h0strip = []
        for i in range(n_fo):
            t_h = apool.tile([fo_sizes[i], SHARD], BF16,
                             name=f"h0strip{i}")
            h0strip.append(t_h)
        with tc.tile_pool(name="phA", bufs=2) as tpool:
            nxt_ch = 0
            for s in range(NSUB):
                xts = []
                for k in range(n_fi):
                    xk = tpool.tile([128, SUB], BF16, tag="xts",
                                    name=f"xts{k}", bufs=3)
                    nc.scalar.dma_start(
                        out=xk[:],
                        in_=xt_d[128 * k:128 * (k + 1),
                                 SUB * s:SUB * (s + 1)])
                    xts.append(xk)
                for i in range(n_fo):
                    ps_h = psA.tile([fo_sizes[i], SUB], F32, tag="ph")
                    for k in range(n_fi):
                        nc.tensor.matmul(
                            out=ps_h[:],
                            lhsT=fcw_sb[k][:, 128 * i:128 * i
                                           + fo_sizes[i]],
                            rhs=xts[k][:],
                            start=(k == 0), stop=(k == n_fi - 1),
                        )
                    nc.vector.tensor_scalar(
                        out=h0strip[i][:, SUB * s:SUB * (s + 1)],
                        in0=ps_h[:],
                        scalar1=fcb_sb[:fo_sizes[i], i:i + 1],
                        scalar2=0.0,
                        op0=AluOp.add, op1=AluOp.max,
                    )
                end = SUB * (s + 1)
                while (nxt_ch + 1) * 128 <= end or (
                        s == NSUB - 1 and nxt_ch < NCH):
                    ch = nxt_ch
                    nxt_ch += 1
                    cw = min(128, SHARD - 128 * ch)
                    ps_z = psB.tile([128, HID2], F32, tag="b")
                    for i in range(n_fo):
                        nc.tensor.matmul(
                            out=ps_z[:cw, :],
                            lhsT=h0strip[i][:, 128 * ch:128 * ch + cw],
                            rhs=w1_sb[i][:],
                            start=(i == 0), stop=(i == n_fo - 1),
                        )
                    zrow = tpool.tile([128, TAB1_W], BF16, tag="zrow",
                                      name="zrow", bufs=3)
                    nc.scalar.mul(out=zrow[:cw, :HID2], in_=ps_z[:cw, :],
                                  mul=dinv_own[:cw, ch:ch + 1])
                    nc.sync.dma_start(
                        out=ztab1_mine[128 * ch:128 * ch + cw, :],
                        in_=zrow[:cw, :])

        rg = [list(range(NCORES))]
        ztab1_full = nc.dram_tensor("ztab1_full", [N, TAB1_W], BF16,
                                    addr_space="Shared")
        cc1i = nc.gpsimd.collective_compute(
            "AllGather", AluOp.bypass, replica_groups=rg,
            ins=[ztab1_mine.opt()], outs=[ztab1_full.ap()[:]],
        )
        cc1 = [cc1i.ins]

        # deferred: sqd row layout (PE op; avoid head-of-line pre-phA)
        ps_tr = psT.tile([NCH, 128], BF16, tag="tr")
        nc.tensor.transpose(out=ps_tr[:], in_=sqd_own[:],
                            identity=ident[:])
        sqd_rows = cpool.tile([NCH, 128], BF16)
        nc.vector.tensor_copy(out=sqd_rows[:], in_=ps_tr[:])
        sqdT = cpool.tile([1, 128 * NCH], BF16)
        for j in range(NCH):
            nc.sync.dma_start(out=sqdT[:, 128 * j:128 * (j + 1)],
                              in_=sqd_rows[j:j + 1, :])

        if stage < 14:
            _dummy_out(nc, wpool, out_d)
            return

        # ---------------- gather helper -------------------------------
        def emit_gather(gtiles, table, width, ccdeps, idx_sb, seq_tiles,
                        tag, bufs):
            g = len(gtiles)
            t0 = 8 * g
            nt = min(8, seq_tiles - t0)
            graw = gpool.tile([128, 8 * width], BF16, tag=tag,
                              name=f"g{tag}{width}_{g}", bufs=bufs)
            sub = graw[:, :nt * width].rearrange("p (t f) -> p t f",
                                                 f=width)
            if isinstance(table, bass.DRamTensorHandle):
                table_ap = table.ap()
            elif isinstance(table, bass.AP):
                table_ap = table
            else:
                table_ap = table[:]
            gi = nc.gpsimd.dma_gather(
                sub, table_ap, idx_sb[:, 8 * t0:8 * (t0 + nt)],
                nt * 128, nt * 128, width, queue_num=g % 4)
            for cc in ccdeps:
                tile.add_dep_helper(gi.ins, cc,
                                    reason="gather reads AllGather table")
            gtiles.append(graw)

        # ---------------- layer 1 + z2 --------------------------------
        l1_sb = apool.tile([128, NCH, HID2], BF16)
        ztab2_mine = dpool.tile([SHARD, OUT_FT], BF16)
        rg1 = []
        while len(rg1) * 8 < net:
            emit_gather(rg1, ztab1_full, TAB1_W, cc1, egidx_sb, net,
                        "gr1", 8)

        for j in range(NCH):
            cw = min(128, SHARD - 128 * j)
            zself = wpool.tile([128, TAB1_W], BF16, tag="zself1",
                               bufs=2)
            nc.sync.dma_start(
                out=zself[:cw, :],
                in_=ztab1_mine[128 * j:128 * j + cw, :])
            ps_a = psB.tile([128, HID2], F32, tag="b")
            for t in range(etiles[j]):
                seq = int(eoff[j]) + t
                nc.tensor.matmul(
                    out=ps_a[:],
                    lhsT=selw_sb[:, 128 * seq:128 * (seq + 1)],
                    rhs=rg1[seq // 8][:, (seq % 8) * TAB1_W:
                                      (seq % 8) * TAB1_W + HID2],
                    start=(t == 0), stop=False,
                )
            nc.tensor.matmul(
                out=ps_a[:], lhsT=ident[:cw, :],
                rhs=zself[:cw, :HID2],
                start=False, stop=False,
            )
            nc.tensor.matmul(
                out=ps_a[:],
                lhsT=sqdT[:, 128 * j:128 * (j + 1)],
                rhs=b1_sb[:], start=False, stop=True,
            )
            nc.scalar.activation(out=l1_sb[:, j, :], in_=ps_a[:],
                                 func=ActFn.Relu,
                                 scale=dinv_own[:, j:j + 1])
            # ---- z2 for chunk j (interleaved) ----
            l1T = []
            for i in range(n_k2):
                ps_tr2 = psT.tile([128, 128], BF16, tag="tr")
                nc.tensor.transpose(
                    out=ps_tr2[:k2_sizes[i], :],
                    in_=l1_sb[:, j, 128 * i:128 * i + k2_sizes[i]],
                    identity=ident[:],
                )
                lt2 = wpool.tile([128, 128], BF16, tag="l1T")
                nc.vector.tensor_copy(out=lt2[:k2_sizes[i], :],
                                      in_=ps_tr2[:k2_sizes[i], :])
                l1T.append(lt2)
            ps_z2 = psB.tile([128, OUT_FT], F32, tag="b")
            for i in range(n_k2):
                nc.tensor.matmul(
                    out=ps_z2[:],
                    lhsT=l1T[i][:k2_sizes[i], :],
                    rhs=w2_sb[i][:],
                    start=(i == 0), stop=(i == n_k2 - 1),
                )
            zrow2 = wpool.tile([128, OUT_FT], BF16, tag="zrow2",
                               bufs=3)
            nc.scalar.mul(out=zrow2[:], in_=ps_z2[:],
                          mul=dinv_own[:, j:j + 1])
            nc.sync.dma_start(
                out=ztab2_mine[128 * j:128 * j + cw, :],
                in_=zrow2[:cw, :])

        if stage < 40:
            for j in range(NCH):
                cw = min(128, SHARD - 128 * j)
                o_sb = wpool.tile([128, OUT_FT], F32, tag="osb")
                nc.scalar.copy(out=o_sb[:], in_=l1_sb[:, j, :OUT_FT])
                nc.sync.dma_start(
                    out=out_d[128 * j:128 * j + cw, :],
                    in_=o_sb[:cw, :])
            return

        ztab2_full = nc.dram_tensor("ztab2_full", [N, OUT_FT], BF16,
                                    addr_space="Shared")
        cc2i = nc.gpsimd.collective_compute(
            "AllGather", AluOp.bypass, replica_groups=rg,
            ins=[ztab2_mine.opt()], outs=[ztab2_full.ap()[:]],
        )
        cc2 = [cc2i.ins]

        if stage < 50:
            _dummy_out(nc, wpool, out_d)
            return

        # ---------------- layer-2 aggregation -------------------------
        # Pass 1 (overlaps the AllGather): self rows via static DMA
        # from ztab2_mine + local-src gathers; partial sums -> SBUF.
        l2acc = apool.tile([128, NCH, OUT_FT], BF16)
        for j in range(NCH):
            cw = min(128, SHARD - 128 * j)
            zself = wpool.tile([128, OUT_FT], BF16, tag="zself",
                               bufs=2)
            nc.sync.dma_start(
                out=zself[:cw, :],
                in_=ztab2_mine[128 * j:128 * j + cw, :])
            ps_l = psB.tile([128, OUT_FT], F32, tag="b")
            nc.tensor.matmul(
                out=ps_l[:], lhsT=ident[:cw, :], rhs=zself[:cw, :],
                start=True, stop=False,
            )
            nc.tensor.matmul(
                out=ps_l[:],
                lhsT=sqdT[:, 128 * j:128 * (j + 1)],
                rhs=b2_sb[:], start=False, stop=True,
            )
            nc.vector.tensor_copy(out=l2acc[:, j, :], in_=ps_l[:])

        # Pass 2: the edge-tile stream from the AllGathered table.
        rg2 = []
        for j in range(NCH):
            et_ = etiles[j]
            cw = min(128, SHARD - 128 * j)
            while len(rg2) * 8 < int(eoff[j]) + et_:
                emit_gather(rg2, ztab2_full, OUT_FT, cc2, egidx_sb,
                            net, "gr2", 8)
            ps_a2 = psB.tile([128, OUT_FT], F32, tag="b")
            for t in range(et_):
                seq = int(eoff[j]) + t
                nc.tensor.matmul(
                    out=ps_a2[:],
                    lhsT=selw_sb[:, 128 * seq:128 * (seq + 1)],
                    rhs=rg2[seq // 8][:, (seq % 8) * OUT_FT:
                                      (seq % 8 + 1) * OUT_FT],
                    start=(t == 0), stop=(t == et_ - 1),
                )
            o_f32 = wpool.tile([128, OUT_FT], F32, tag="of")
            nc.vector.tensor_tensor(
                out=o_f32[:], in0=ps_a2[:], in1=l2acc[:, j, :],
                op=AluOp.add)
            o_sb = wpool.tile([128, OUT_FT], F32, tag="osb")
            nc.scalar.activation(out=o_sb[:], in_=o_f32[:],
                                 func=ActFn.Relu,
                                 scale=dinv_own[:, j:j + 1])
            nc.sync.dma_start(out=out_d[128 * j:128 * j + cw, :],
                              in_=o_sb[:cw, :])

    with tile.TileContext(nc) as tc:
        with (
            tc.tile_pool(name="const", bufs=1) as cpool,
            tc.tile_pool(name="acts", bufs=1) as apool,
            tc.tile_pool(name="gath", bufs=1) as gpool,
            tc.tile_pool(name="work", bufs=2) as wpool,
            tc.tile_pool(name="psA", bufs=2, space="PSUM") as psA,
            tc.tile_pool(name="psB", bufs=3, space="PSUM") as psB,
            tc.tile_pool(name="psT", bufs=2, space="PSUM") as psT,
            tc.tile_pool(name="dram", bufs=1, space="DRAM") as dpool,
        ):
            _emit(tc, cpool, apool, gpool, wpool, psA, psB, psT, dpool)
    nc.compile()
    _fix_multiwait(nc)
    return nc


# --------------------------------------------------------------------------
# Entry point
# --------------------------------------------------------------------------

_NC_CACHE = {}


def kernel(x, edge_index, edge_attr, fc_W, fc_b, W1, b1, W2, b2,
           _trace=False):
    meta, ep, in_edges = _prep_edges(edge_index, edge_attr)
    key = (tuple(meta["etiles"]), meta["K2"])
    if key not in _NC_CACHE:
        _NC_CACHE[key] = build_nc(meta, ep)
    nc = _NC_CACHE[key]

    x = np.asarray(x, np.float32)
    bf = ml_dtypes.bfloat16
    shared = {
        "fcw": np.asarray(fc_W, np.float32).astype(bf),
        "fcb": np.asarray(fc_b, np.float32).reshape(HID1, 1),
        "w1": np.asarray(W1, np.float32).astype(bf),
        "b1": np.asarray(b1, np.float32).reshape(1, HID2).astype(bf),
        "w2": np.asarray(W2, np.float32).astype(bf),
        "b2": np.asarray(b2, np.float32).reshape(1, OUT_FT).astype(bf),
    }
    in_maps = []
    for c in range(NCORES):
        xt = np.ascontiguousarray(
            x[c * SHARD:(c + 1) * SHARD, :].T).astype(bf)
        in_maps.append({"xt": xt, **in_edges[c], **shared})

    res = run_bass_kernel_spmd(nc, in_maps, list(range(NCORES)),
                               trace=_trace)
    out = np.concatenate([res.results[c]["out"] for c in range(NCORES)],
                         axis=0)
    if _trace:
        kernel._last_exec_time_ns = res.exec_time_ns
        kernel._last_results = res
    return out


# revision 24
# speedup vs baseline: 1.0677x; 1.0558x over previous
"""GCN encoder (Linear+ReLU -> GCNConv+ReLU -> GCNConv -> ReLU) on 8 TRN2
NeuronCores.

Architecture (v7): fully node-sharded with two 8-rank AllGathers.
  - Core c computes z1 = dinv*(relu(x_c @ fc_W + fc_b) @ W1) for its own
    2500 nodes only (~30us of PE) and AllGathers the bf16 table.  The
    collective-runtime barrier (~40-65us warm) elapses during the early
    compute, so AG1 starts almost immediately after z1.
  - Degrees come from a single DVE reduce over a compact host layout of
    the own-shard edge weights (w at [dst%128, dst//128, k]) - no
    one-hot matmul pass and no cross-core exchange.
  - Per-chunk tile segments [self][local][remote].  Self rows are read
    with a static DMA from the own-shard table (identity stationary, no
    selw entry, no gather).  Local (in-shard src) tiles gather from the
    own table before the AllGather lands; remote tiles gather from the
    AllGathered table.  The same index arrays serve both layers.
  - z2 is produced per-chunk inside the layer-1 loop; during the z2
    AllGather the layer-2 self/local partial sums accumulate into SBUF.

Host-side preprocessing is index manipulation / data layout only.  All
arithmetic (degree sums, rsqrt, matmuls, aggregation) runs on device.
"""

import os

import numpy as np
import ml_dtypes

import concourse.bacc as bacc
import concourse.bass as bass
import concourse.mybir as mybir
import concourse.tile as tile
from concourse.bass_utils import run_bass_kernel_spmd
from concourse.masks import make_identity

F32 = mybir.dt.float32
BF16 = mybir.dt.bfloat16
I16 = mybir.dt.int16

N = 20000
E = 320000
IN_FT, HID1, HID2, OUT_FT = 256, 400, 200, 128
NCORES = 8
SHARD = N // NCORES            # 2500 nodes per core
NCH = (SHARD + 127) // 128     # 20 local dst chunks per core (last 68)
TAB1_W = 256                   # padded row width of layer-1 gather table
AluOp = mybir.AluOpType
ActFn = mybir.ActivationFunctionType


def _cdiv(a, b):
    return (a + b - 1) // b


# --------------------------------------------------------------------------
# Host-side sharding / layout
# --------------------------------------------------------------------------

def _idx_layout(a):
    g = a.astype(np.int16).reshape(-1, 16).T.copy()
    return np.ascontiguousarray(np.tile(g, (8, 1)))


def _prep_edges(edge_index, edge_attr):
    """Partition edges by dst shard into per-chunk segments
    [local(in-shard src)][remote], padded to 128-edge tiles with counts
    maximized across cores (one SPMD program serves all cores).  Self
    loops are NOT in the edge lists (identity-stationary on the table
    chunk rows).  selw covers local+remote tiles."""
    src = np.ascontiguousarray(edge_index[0]).astype(np.int64)
    dst = np.ascontiguousarray(edge_index[1]).astype(np.int64)
    w_all = np.ascontiguousarray(edge_attr).astype(np.float32)

    per_core = []
    ecounts = np.zeros((NCORES, NCH), np.int64)
    for c in range(NCORES):
        lo, hi = c * SHARD, (c + 1) * SHARD
        m = (dst >= lo) & (dst < hi)
        s = src[m]
        d = dst[m] - lo
        w = w_all[m]
        o = np.argsort(d >> 7, kind="stable")
        s, d, w = s[o], d[o], w[o]
        ch = d >> 7
        for j in range(NCH):
            ecounts[c, j] = int((ch == j).sum())
        per_core.append((s, d, w, lo))

    etiles = [max(1, _cdiv(int(ecounts[:, j].max()), 128))
              for j in range(NCH)]
    net = int(np.sum(etiles))
    nsel = net
    ep = 128 * nsel
    eoff = np.concatenate([[0], np.cumsum(etiles)])[:-1]

    # per-core own-degree layout sized by the global max per-dst count
    K2 = 0
    for c in range(NCORES):
        lo = c * SHARD
        m = (dst >= lo) & (dst < lo + SHARD)
        dl = dst[m] - lo
        cnt = np.zeros((NCH, 128), np.int64)
        np.add.at(cnt, (dl >> 7, dl & 127), 1)
        K2 = max(K2, int(cnt.max()))

    in_edges = []
    for c in range(NCORES):
        s, d, w, lo = per_core[c]
        ch = d >> 7
        w_pad = np.zeros(ep, np.float32)
        dslot_pad = np.zeros(ep, np.int64)
        esrc = np.zeros(128 * net, np.int64)
        for j in range(NCH):
            mm = ch == j
            se, de, we = s[mm], d[mm], w[mm]
            ge = 128 * int(eoff[j])
            dslot_pad[ge:ge + len(se)] = de - 128 * j
            w_pad[ge:ge + len(se)] = we
            esrc[ge:ge + len(se)] = se          # global rows
        selw = np.zeros((ep, 128), np.float32)
        selw[np.arange(ep), dslot_pad] = w_pad
        selw_pack = (
            selw.reshape(nsel, 128, 128)
            .transpose(1, 0, 2)
            .reshape(128, nsel * 128)
            .astype(ml_dtypes.bfloat16)
        )
        degw_own = np.zeros((128, NCH, K2), np.float32)
        m = (dst >= lo) & (dst < lo + SHARD)
        dl_all = dst[m] - lo
        wl_all = w_all[m]
        kfill2 = np.zeros((NCH, 128), np.int64)
        lp, lc = dl_all & 127, dl_all >> 7
        for i in range(len(dl_all)):
            p, chn = int(lp[i]), int(lc[i])
            degw_own[p, chn, kfill2[chn, p]] = wl_all[i]
            kfill2[chn, p] += 1

        in_edges.append({
            "selw": selw_pack,
            "egidx": _idx_layout(esrc),
            "degw_own": degw_own.reshape(128, -1),
        })
    meta = dict(etiles=etiles, eoff=eoff, net=net, K2=K2)
    return meta, ep, in_edges


# --------------------------------------------------------------------------
# Device program
# --------------------------------------------------------------------------

def _fix_multiwait(nc):
    """This neuronxcc build only accepts ONE sync-wait on non-EventSemaphore
    instructions; bacc's splitter allows two on DMAs.  Move excess waits onto
    inserted EventSemaphore NOPs (2 waits each) preceding the instruction."""
    nev = 0
    for bb in nc.main_func.blocks:
        changed = False
        out = []
        for ins in bb.instructions:
            si = ins.sync_info
            waits = list(si.on_wait) if si and si.on_wait else []
            limit = 2 if isinstance(ins, mybir.InstEventSemaphore) else 1
            if len(waits) > limit:
                extra, keep = waits[:-limit], waits[-limit:]
                for i in range(0, len(extra), 2):
                    ev = mybir.InstEventSemaphore(
                        name=f"{ins.name}-evw{i}", ins=[], outs=[])
                    ev.engine = ins.engine
                    ev.sync_info = mybir.SyncInfo(
                        on_wait=extra[i:i + 2], on_update=[])
                    out.append(ev)
                    nev += 1
                si.on_wait = keep
                changed = True
            out.append(ins)
        if changed:
            bb.instructions = out
    return nev


def _dummy_out(nc, wpool, out_d):
    for j in range(NCH):
        cw = min(128, SHARD - 128 * j)
        o_sb = wpool.tile([128, OUT_FT], F32, tag="osb")
        nc.vector.memset(o_sb[:], 0.0)
        nc.sync.dma_start(out=out_d[128 * j:128 * j + cw, :],
                          in_=o_sb[:cw, :])


def build_nc(meta, ep):
    stage = int(os.environ.get("K_STAGE", "50"))
    etiles, eoff = meta["etiles"], meta["eoff"]
    net, K2 = meta["net"], meta["K2"]
    nsel = net
    assert ep == 128 * nsel
    nc = bacc.Bacc("TRN2", target_bir_lowering=False, debug=False,
                   num_devices=NCORES, num_swdge_queues=4)

    xt_d = nc.dram_tensor("xt", [IN_FT, SHARD], BF16,
                          kind="ExternalInput")
    selw_d = nc.dram_tensor("selw", [128, ep], BF16, kind="ExternalInput")
    egidx_d = nc.dram_tensor("egidx", [128, 8 * net], I16,
                             kind="ExternalInput")
    degwo_d = nc.dram_tensor("degw_own", [128, NCH * K2], F32,
                             kind="ExternalInput")
    fcw_d = nc.dram_tensor("fcw", [IN_FT, HID1], BF16,
                           kind="ExternalInput")
    fcb_d = nc.dram_tensor("fcb", [HID1, 1], F32, kind="ExternalInput")
    w1_d = nc.dram_tensor("w1", [HID1, HID2], BF16, kind="ExternalInput")
    b1_d = nc.dram_tensor("b1", [1, HID2], BF16, kind="ExternalInput")
    w2_d = nc.dram_tensor("w2", [HID2, OUT_FT], BF16,
                          kind="ExternalInput")
    b2_d = nc.dram_tensor("b2", [1, OUT_FT], BF16, kind="ExternalInput")
    out_d = nc.dram_tensor("out", [SHARD, OUT_FT], F32, kind="ExternalOutput")

    n_fi = _cdiv(IN_FT, 128)     # 2
    n_fo = _cdiv(HID1, 128)      # 4 (128,128,128,16)
    n_k2 = _cdiv(HID2, 128)      # 2 (128,72)
    fo_sizes = [min(128, HID1 - 128 * i) for i in range(n_fo)]
    k2_sizes = [min(128, HID2 - 128 * i) for i in range(n_k2)]
    NSUB = 5
    SUB = SHARD // NSUB          # 500

    def _emit(tc, cpool, apool, gpool, wpool, psA, psB, psT, dpool):
        # ---------------- early inputs ----------------
        degwo_sb = cpool.tile([128, NCH * K2], F32)
        nc.sync.dma_start(out=degwo_sb[:], in_=degwo_d[:])

        fcw_sb = []
        for i in range(n_fi):
            t = cpool.tile([128, HID1], BF16, name=f"fcw{i}")
            nc.scalar.dma_start(out=t[:],
                                in_=fcw_d[128 * i:128 * (i + 1), :])
            fcw_sb.append(t)
        w1_sb = []
        for i in range(n_fo):
            t = cpool.tile([fo_sizes[i], HID2], BF16, name=f"w1_{i}")
            nc.scalar.dma_start(
                out=t[:], in_=w1_d[128 * i:128 * i + fo_sizes[i], :])
            w1_sb.append(t)
        w2_sb = []
        for i in range(n_k2):
            t = cpool.tile([k2_sizes[i], OUT_FT], BF16, name=f"w2_{i}")
            nc.scalar.dma_start(
                out=t[:], in_=w2_d[128 * i:128 * i + k2_sizes[i], :])
            w2_sb.append(t)
        fcb_sb = cpool.tile([128, n_fo], F32, name="fcb_sb")
        for i in range(n_fo):
            nc.sync.dma_start(
                out=fcb_sb[:fo_sizes[i], i:i + 1],
                in_=fcb_d[128 * i:128 * i + fo_sizes[i], :])
        b1_sb = cpool.tile([1, HID2], BF16)
        nc.scalar.dma_start(out=b1_sb[:], in_=b1_d[:])
        b2_sb = cpool.tile([1, OUT_FT], BF16)
        nc.scalar.dma_start(out=b2_sb[:], in_=b2_d[:])

        ident = cpool.tile([128, 128], BF16)
        make_identity(nc, ident[:])

        # ---------------- degrees / normalization (DVE) ---------------
        deg_own = cpool.tile([128, NCH], F32)
        nc.vector.tensor_reduce(
            out=deg_own[:],
            in_=degwo_sb[:].rearrange("p (c k) -> p c k", k=K2),
            axis=mybir.AxisListType.X, op=AluOp.add)
        nc.vector.tensor_scalar_add(deg_own[:], deg_own[:], 1.0)
        dinv_own = cpool.tile([128, NCH], F32)
        nc.vector.reciprocal(out=dinv_own[:], in_=deg_own[:])
        nc.scalar.activation(out=dinv_own[:], in_=dinv_own[:],
                             func=ActFn.Sqrt)
        sqd_own = cpool.tile([128, NCH], BF16)
        nc.scalar.activation(out=sqd_own[:], in_=deg_own[:],
                             func=ActFn.Sqrt)

        # selw / idx loads stream in behind the small early loads
        selw_sb = cpool.tile([128, ep], BF16)
        for s0 in range(0, nsel, 44):
            s1 = min(nsel, s0 + 44)
            nc.sync.dma_start(out=selw_sb[:, 128 * s0:128 * s1],
                              in_=selw_d[:, 128 * s0:128 * s1])
        egidx_sb = cpool.tile([128, 8 * net], I16)
        nc.sync.dma_start(out=egidx_sb[:], in_=egidx_d[:])

        if stage < 12:
            _dummy_out(nc, wpool, out_d)
            return

        # ---------------- phase A + z1 (own shard) -> own table -------
        ztab1_mine = dpool.tile([SHARD, TAB1_W], BF16)
        h0strip = []
        for i in range(n_fo):
            t_h = apool.tile([fo_sizes[i], SHARD], BF16,
                             name=f"h0strip{i}")
            h0strip.append(t_h)
        with tc.tile_pool(name="phA", bufs=2) as tpool:
            nxt_ch = 0
            for s in range(NSUB):
                xts = []
                for k in range(n_fi):
                    xk = tpool.tile([128, SUB], BF16, tag="xts",
                                    name=f"xts{k}", bufs=3)
                    nc.scalar.dma_start(
                        out=xk[:],
                        in_=xt_d[128 * k:128 * (k + 1),
                                 SUB * s:SUB * (s + 1)])
                    xts.append(xk)
                for i in range(n_fo):
                    ps_h = psA.tile([fo_sizes[i], SUB], F32, tag="ph")
                    for k in range(n_fi):
                        nc.tensor.matmul(
                            out=ps_h[:],
                            lhsT=fcw_sb[k][:, 128 * i:128 * i
                                           + fo_sizes[i]],
                            rhs=xts[k][:],
                            start=(k == 0), stop=(k == n_fi - 1),
                        )
                    nc.vector.tensor_scalar(
                        out=h0strip[i][:, SUB * s:SUB * (s + 1)],
                        in0=ps_h[:],
                        scalar1=fcb_sb[:fo_sizes[i], i:i + 1],
                        scalar2=0.0,
                        op0=AluOp.add, op1=AluOp.max,
                    )
                end = SUB * (s + 1)
                while (nxt_ch + 1) * 128 <= end or (
                        s == NSUB - 1 and nxt_ch < NCH):
                    ch = nxt_ch
                    nxt_ch += 1
                    cw = min(128, SHARD - 128 * ch)
                    ps_z = psB.tile([128, HID2], F32, tag="b")
                    for i in range(n_fo):
                        nc.tensor.matmul(
                            out=ps_z[:cw, :],
                            lhsT=h0strip[i][:, 128 * ch:128 * ch + cw],
                            rhs=w1_sb[i][:],
                            start=(i == 0), stop=(i == n_fo - 1),
                        )
                    zrow = tpool.tile([128, TAB1_W], BF16, tag="zrow",
                                      name="zrow", bufs=3)
                    nc.scalar.mul(out=zrow[:cw, :HID2], in_=ps_z[:cw, :],
                                  mul=dinv_own[:cw, ch:ch + 1])
                    nc.sync.dma_start(
                        out=ztab1_mine[128 * ch:128 * ch + cw, :],
                        in_=zrow[:cw, :])

        rg = [list(range(NCORES))]
        ztab1_full = nc.dram_tensor("ztab1_full", [N, TAB1_W], BF16,
                                    addr_space="Shared")
        cc1i = nc.gpsimd.collective_compute(
            "AllGather", AluOp.bypass, replica_groups=rg,
            ins=[ztab1_mine.opt()], outs=[ztab1_full.ap()[:]],
        )
        cc1 = [cc1i.ins]

        # deferred: sqd row layout (PE op; avoid head-of-line pre-phA)
        ps_tr = psT.tile([NCH, 128], BF16, tag="tr")
        nc.tensor.transpose(out=ps_tr[:], in_=sqd_own[:],
                            identity=ident[:])
        sqd_rows = cpool.tile([NCH, 128], BF16)
        nc.vector.tensor_copy(out=sqd_rows[:], in_=ps_tr[:])
        sqdT = cpool.tile([1, 128 * NCH], BF16)
        for j in range(NCH):
            nc.sync.dma_start(out=sqdT[:, 128 * j:128 * (j + 1)],
                              in_=sqd_rows[j:j + 1, :])

        if stage < 14:
            _dummy_out(nc, wpool, out_d)
            return

        # ---------------- gather helper -------------------------------
        def emit_gather(gtiles, table, width, ccdeps, idx_sb, seq_tiles,
                        tag, bufs):
            g = len(gtiles)
            t0 = 8 * g
            nt = min(8, seq_tiles - t0)
            graw = gpool.tile([128, 8 * width], BF16, tag=tag,
                              name=f"g{tag}{width}_{g}", bufs=bufs)
            sub = graw[:, :nt * width].rearrange("p (t f) -> p t f",
                                                 f=width)
            if isinstance(table, bass.DRamTensorHandle):
                table_ap = table.ap()
            elif isinstance(table, bass.AP):
                table_ap = table
            else:
                table_ap = table[:]
            gi = nc.gpsimd.dma_gather(
                sub, table_ap, idx_sb[:, 8 * t0:8 * (t0 + nt)],
                nt * 128, nt * 128, width, queue_num=g % 4)
            for cc in ccdeps:
                tile.add_dep_helper(gi.ins, cc,
                                    reason="gather reads AllGather table")
            gtiles.append(graw)

        # ---------------- layer 1 + z2 --------------------------------
        l1_sb = apool.tile([128, NCH, HID2], BF16)
        ztab2_mine = dpool.tile([SHARD, OUT_FT], BF16)
        rg1 = []
        while len(rg1) * 8 < net:
            emit_gather(rg1, ztab1_full, TAB1_W, cc1, egidx_sb, net,
                        "gr1", 10)

        for j in range(NCH):
            cw = min(128, SHARD - 128 * j)
            zself = wpool.tile([128, TAB1_W], BF16, tag="zself1",
                               bufs=2)
            nc.sync.dma_start(
                out=zself[:cw, :],
                in_=ztab1_mine[128 * j:128 * j + cw, :])
            ps_a = psB.tile([128, HID2], F32, tag="b")
            for t in range(etiles[j]):
                seq = int(eoff[j]) + t
                nc.tensor.matmul(
                    out=ps_a[:],
                    lhsT=selw_sb[:, 128 * seq:128 * (seq + 1)],
                    rhs=rg1[seq // 8][:, (seq % 8) * TAB1_W:
                                      (seq % 8) * TAB1_W + HID2],
                    start=(t == 0), stop=False,
                )
            nc.tensor.matmul(
                out=ps_a[:], lhsT=ident[:cw, :],
                rhs=zself[:cw, :HID2],
                start=False, stop=False,
            )
            nc.tensor.matmul(
                out=ps_a[:],
                lhsT=sqdT[:, 128 * j:128 * (j + 1)],
                rhs=b1_sb[:], start=False, stop=True,
            )
            nc.scalar.activation(out=l1_sb[:, j, :], in_=ps_a[:],
                                 func=ActFn.Relu,
                                 scale=dinv_own[:, j:j + 1])
            # ---- z2 for chunk j (interleaved) ----
            l1T = []
            for i in range(n_k2):
                ps_tr2 = psT.tile([128, 128], BF16, tag="tr")
                nc.tensor.transpose(
                    out=ps_tr2[:k2_sizes[i], :],
                    in_=l1_sb[:, j, 128 * i:128 * i + k2_sizes[i]],
                    identity=ident[:],
                )
                lt2 = wpool.tile([128, 128], BF16, tag="l1T")
                nc.vector.tensor_copy(out=lt2[:k2_sizes[i], :],
                                      in_=ps_tr2[:k2_sizes[i], :])
                l1T.append(lt2)
            ps_z2 = psB.tile([128, OUT_FT], F32, tag="b")
            for i in range(n_k2):
                nc.tensor.matmul(
                    out=ps_z2[:],
                    lhsT=l1T[i][:k2_sizes[i], :],
                    rhs=w2_sb[i][:],
                    start=(i == 0), stop=(i == n_k2 - 1),
                )
            zrow2 = wpool.tile([128, OUT_FT], BF16, tag="zrow2",
                               bufs=3)
            nc.scalar.mul(out=zrow2[:], in_=ps_z2[:],
                          mul=dinv_own[:, j:j + 1])
            nc.sync.dma_start(
                out=ztab2_mine[128 * j:128 * j + cw, :],
                in_=zrow2[:cw, :])

        if stage < 40:
            for j in range(NCH):
                cw = min(128, SHARD - 128 * j)
                o_sb = wpool.tile([128, OUT_FT], F32, tag="osb")
                nc.scalar.copy(out=o_sb[:], in_=l1_sb[:, j, :OUT_FT])
                nc.sync.dma_start(
                    out=out_d[128 * j:128 * j + cw, :],
                    in_=o_sb[:cw, :])
            return

        ztab2_full = nc.dram_tensor("ztab2_full", [N, OUT_FT], BF16,
                                    addr_space="Shared")
        cc2i = nc.gpsimd.collective_compute(
            "AllGather", AluOp.bypass, replica_groups=rg,
            ins=[ztab2_mine.opt()], outs=[ztab2_full.ap()[:]],
        )
        cc2 = [cc2i.ins]

        if stage < 50:
            _dummy_out(nc, wpool, out_d)
            return

        # ---------------- layer-2 aggregation -------------------------
        # Pass 1 (overlaps the AllGather): self rows via static DMA
        # from ztab2_mine + local-src gathers; partial sums -> SBUF.
        l2acc = apool.tile([128, NCH, OUT_FT], BF16)
        for j in range(NCH):
            cw = min(128, SHARD - 128 * j)
            zself = wpool.tile([128, OUT_FT], BF16, tag="zself",
                               bufs=2)
            nc.sync.dma_start(
                out=zself[:cw, :],
                in_=ztab2_mine[128 * j:128 * j + cw, :])
            ps_l = psB.tile([128, OUT_FT], F32, tag="b")
            nc.tensor.matmul(
                out=ps_l[:], lhsT=ident[:cw, :], rhs=zself[:cw, :],
                start=True, stop=False,
            )
            nc.tensor.matmul(
                out=ps_l[:],
                lhsT=sqdT[:, 128 * j:128 * (j + 1)],
                rhs=b2_sb[:], start=False, stop=True,
            )
            nc.vector.tensor_copy(out=l2acc[:, j, :], in_=ps_l[:])

        # Pass 2: the edge-tile stream from the AllGathered table.
        rg2 = []
        for j in range(NCH):
            et_ = etiles[j]
            cw = min(128, SHARD - 128 * j)
            while len(rg2) * 8 < int(eoff[j]) + et_:
                emit_gather(rg2, ztab2_full, OUT_FT, cc2, egidx_sb,
                            net, "gr2", 10)
            ps_a2 = psB.tile([128, OUT_FT], F32, tag="b")
            for t in range(et_):
                seq = int(eoff[j]) + t
                nc.tensor.matmul(
                    out=ps_a2[:],
                    lhsT=selw_sb[:, 128 * seq:128 * (seq + 1)],
                    rhs=rg2[seq // 8][:, (seq % 8) * OUT_FT:
                                      (seq % 8 + 1) * OUT_FT],
                    start=(t == 0), stop=(t == et_ - 1),
                )
            o_f32 = wpool.tile([128, OUT_FT], F32, tag="of")
            nc.vector.tensor_tensor(
                out=o_f32[:], in0=ps_a2[:], in1=l2acc[:, j, :],
                op=AluOp.add)
            o_sb = wpool.tile([128, OUT_FT], F32, tag="osb")
            nc.scalar.activation(out=o_sb[:], in_=o_f32[:],
                                 func=ActFn.Relu,
                                 scale=dinv_own[:, j:j + 1])
            nc.sync.dma_start(out=out_d[128 * j:128 * j + cw, :],
                              in_=o_sb[:cw, :])

    with tile.TileContext(nc) as tc:
        with (
            tc.tile_pool(name="const", bufs=1) as cpool,
            tc.tile_pool(name="acts", bufs=1) as apool,
            tc.tile_pool(name="gath", bufs=1) as gpool,
            tc.tile_pool(name="work", bufs=2) as wpool,
            tc.tile_pool(name="psA", bufs=2, space="PSUM") as psA,
            tc.tile_pool(name="psB", bufs=3, space="PSUM") as psB,
            tc.tile_pool(name="psT", bufs=2, space="PSUM") as psT,
            tc.tile_pool(name="dram", bufs=1, space="DRAM") as dpool,
        ):
            _emit(tc, cpool, apool, gpool, wpool, psA, psB, psT, dpool)
    nc.compile()
    _fix_multiwait(nc)
    return nc


# --------------------------------------------------------------------------
# Entry point
# --------------------------------------------------------------------------

_NC_CACHE = {}


def kernel(x, edge_index, edge_attr, fc_W, fc_b, W1, b1, W2, b2,
           _trace=False):
    meta, ep, in_edges = _prep_edges(edge_index, edge_attr)
    key = (tuple(meta["etiles"]), meta["K2"])
    if key not in _NC_CACHE:
        _NC_CACHE[key] = build_nc(meta, ep)
    nc = _NC_CACHE[key]

    x = np.asarray(x, np.float32)
    bf = ml_dtypes.bfloat16
    shared = {
        "fcw": np.asarray(fc_W, np.float32).astype(bf),
        "fcb": np.asarray(fc_b, np.float32).reshape(HID1, 1),
        "w1": np.asarray(W1, np.float32).astype(bf),
        "b1": np.asarray(b1, np.float32).reshape(1, HID2).astype(bf),
        "w2": np.asarray(W2, np.float32).astype(bf),
        "b2": np.asarray(b2, np.float32).reshape(1, OUT_FT).astype(bf),
    }
    in_maps = []
    for c in range(NCORES):
        xt = np.ascontiguousarray(
            x[c * SHARD:(c + 1) * SHARD, :].T).astype(bf)
        in_maps.append({"xt": xt, **in_edges[c], **shared})

    res = run_bass_kernel_spmd(nc, in_maps, list(range(NCORES)),
                               trace=_trace)
    out = np.concatenate([res.results[c]["out"] for c in range(NCORES)],
                         axis=0)
    if _trace:
        kernel._last_exec_time_ns = res.exec_time_ns
        kernel._last_results = res
    return out
